# revision 8
# baseline (speedup 1.0000x reference)
"""Causal self-attention (B=2,T=2048,C=1024,H=16) on 8 trn2 NeuronCores.

Sharding: core c handles batch b=c//4 and 4 heads (c%4)*4..+4 (tensor-parallel
over heads x data-parallel over batch).

All matmuls in bf16 (inputs quantized host-side / on-engine; PSUM stays f32):
  stage A: qkT = (Wqk)^T @ x^T  (scale folded into Wq), V = x @ Wv (+ones col)
  stage B: per (head, kt-pair): S^T tile = K' Q -> exp(s-2) -> causal mask
           (tensor_mul with a precomputed triangular mask on diag tiles only)
  AV transposed: y[qt] += es_kt^T-slice @ V_kt  -> psY [128q, 4qt x 65]
           (65th col = ones -> l); y = psY * (1/l) per-partition -> bf16
  yT via DMA xbar transpose; proj: outT = Wp^T @ yT, PSUM -> DRAM direct.
Host sums the 4 per-batch partials, adds b_proj, transposes back.
"""
import sys

sys.path.insert(0, "/opt/trn_rl_repo")

import numpy as np
import ml_dtypes

import concourse.bass as bass
import concourse.mybir as mybir
import concourse.tile as tile
from concourse import bacc
from concourse.bass_utils import run_bass_kernel_spmd

B, T, C, H, HD = 2, 2048, 1024, 16, 64
NCORES = 8
HPC = 4            # heads per core
CT = C // 128      # 8 contraction tiles
TJ = T // 512      # 4 q chunks
TT = T // 128      # 16 tok tiles
VW = HPC * (HD + 1)  # 260: V cols per core incl. ones column per head
F32 = mybir.dt.float32
BF = mybir.dt.bfloat16
EXP = mybir.ActivationFunctionType.Exp

_CACHE = {}


def _emit(tc, nc, d):
    d_xT, d_wqk, d_wv, d_wp, d_bqk, d_bv, d_mega, d_out = d
    with tc.tile_pool(name="const", bufs=1) as pc, \
         tc.tile_pool(name="qk", bufs=1) as pqk, \
         tc.tile_pool(name="vv", bufs=1) as pvv, \
         tc.tile_pool(name="yt", bufs=1) as pyt:
        bqk = pc.tile([128, 4], F32, tag="bqk")
        bv = pc.tile([128, VW], F32, tag="bv")
        mega = pc.tile([128, 896], BF, tag="mega")
        negtwo = pc.tile([128, 1], F32, tag="negtwo")
        nc.gpsimd.memset(negtwo[:], -2.0)

        qkT = [pqk.tile([128, T], BF, tag=f"qk{i}", name=f"qkT{i}") for i in range(4)]
        V = [pvv.tile([128, VW], BF, tag=f"v{i}", name=f"V{i}") for i in range(TT)]
        yT = [pyt.tile([128, T], BF, tag=f"y{i}", name=f"yT{i}") for i in range(2)]

        # ---------------- stage A: QKV projections ----------------
        with tc.tile_pool(name="w_in", bufs=1) as pw, \
             tc.tile_pool(name="x_in", bufs=1) as px, \
             tc.tile_pool(name="psA", bufs=2, space="PSUM") as psA:
            wqk = pw.tile([128, CT * 512], BF, tag="wqk")
            wv = pw.tile([128, CT * VW], BF, tag="wv")
            xT = px.tile([128, CT * T], BF, tag="xT")
            # first accumulation group (tj=0) needs wqk/xT-tj0 for every ct
            nc.sync.dma_start(wqk[:, :4 * 512], d_wqk[:, :4 * 512])
            for ct in range(CT):
                nc.sync.dma_start(
                    xT[:, ct * T:ct * T + 512], d_xT[:, ct * T:ct * T + 512])
            nc.sync.dma_start(wqk[:, 4 * 512:], d_wqk[:, 4 * 512:])
            nc.sync.dma_start(bqk[:], d_bqk)
            nc.sync.dma_start(wv[:], d_wv)
            nc.sync.dma_start(bv[:], d_bv)
            nc.sync.dma_start(mega[:], d_mega)
            for tj in range(1, TJ):
                for ct in range(CT):
                    nc.sync.dma_start(
                        xT[:, ct * T + tj * 512:ct * T + (tj + 1) * 512],
                        d_xT[:, ct * T + tj * 512:ct * T + (tj + 1) * 512])
            wp = pc.tile([128, 2 * C], BF, tag="wp")
            nc.sync.dma_start(wp[:], d_wp)

            for tj in range(TJ):
                # q,k for 4 heads: M-tiles [q01, k01, q23, k23]
                for mo in range(4):
                    ps = psA.tile([128, 512], F32, tag="psqk")
                    for ct in range(CT):
                        nc.tensor.matmul(
                            ps[:],
                            wqk[:, ct * 512 + mo * 128:ct * 512 + (mo + 1) * 128],
                            xT[:, ct * T + tj * 512:ct * T + (tj + 1) * 512],
                            start=(ct == 0), stop=(ct == CT - 1))
                    nc.vector.tensor_scalar_add(
                        qkT[mo][:, tj * 512:(tj + 1) * 512], ps[:], bqk[:, mo:mo + 1])
                # V natural layout for tok tiles of this chunk
                for tt in range(4 * tj, 4 * tj + 4):
                    psv = psA.tile([128, VW], F32, tag="psv")
                    for ct in range(CT):
                        nc.tensor.matmul(
                            psv[:],
                            xT[:, ct * T + tt * 128:ct * T + (tt + 1) * 128],
                            wv[:, ct * VW:(ct + 1) * VW],
                            start=(ct == 0), stop=(ct == CT - 1))
                    nc.gpsimd.tensor_add(V[tt][:], psv[:], bv[:])

        # ---------------- stage B + C: attention, proj per qj ----------------
        with tc.tile_pool(name="psS", bufs=2, space="PSUM") as psS, \
             tc.tile_pool(name="psY", bufs=2, space="PSUM") as psY, \
             tc.tile_pool(name="psC", bufs=2, space="PSUM") as psC, \
             tc.tile_pool(name="ex", bufs=4) as pex, \
             tc.tile_pool(name="nrm", bufs=4) as pn, \
             tc.tile_pool(name="ysb", bufs=8) as pysb, \
             tc.tile_pool(name="po", bufs=4) as po:
            for qj in range(TJ):
                nkt = 4 * qj + 4
                npair = nkt // 2
                y_sb = [pysb.tile([128, 256], BF, tag="ysb", name=f"ysb{qj}_{q}")
                        for q in range(4)]
                for hp in range(2):
                    qt_, kt_ = qkT[2 * hp], qkT[2 * hp + 1]
                    for lh in range(2):
                        h_loc = 2 * hp + lh
                        lo, hi = 64 * lh, 64 * lh + 64
                        psy = psY.tile([128, 512], F32, tag="psy")
                        first_mm = True
                        for p in range(npair):
                            last = (p == npair - 1)
                            off = 256 if last else 0
                            s = psS.tile([128, 2, 512], F32, tag="s")
                            es = pex.tile([128, 2, 512], BF, tag="es")
                            for ki in range(2):
                                kt = 2 * p + ki
                                nc.tensor.matmul(
                                    s[:, ki, off:512],
                                    kt_[lo:hi, kt * 128:(kt + 1) * 128],
                                    qt_[lo:hi, qj * 512 + off:(qj + 1) * 512],
                                    start=True, stop=True)
                            nc.scalar.activation(
                                es[:, :, off:512], s[:, :, off:512], EXP,
                                bias=negtwo[:])
                            for ki in range(2):
                                kt = 2 * p + ki
                                r = kt - 4 * qj
                                if r >= 0:
                                    # causal mask: keep k<=q <-> p<=c-128r
                                    mo_ = 384 - 128 * r
                                    sl = es[:, ki, off:512]
                                    eng = nc.vector if (kt % 2 == 0) else nc.gpsimd
                                    eng.tensor_mul(
                                        sl, sl, mega[:, mo_ + off:mo_ + 512])
                            # AV (transposed): psy[qt] += es_kt(qt)^T-ish @ V_kt
                            for qt in range(4):
                                for ki in range(2):
                                    kt = 2 * p + ki
                                    if kt > 4 * qj + qt:
                                        continue
                                    nc.tensor.matmul(
                                        psy[:, qt * 128:qt * 128 + 65],
                                        es[:, ki, qt * 128:(qt + 1) * 128],
                                        V[kt][:, h_loc * 65:h_loc * 65 + 65],
                                        start=first_mm,
                                        stop=(kt == 4 * qj + qt and qt == 3),
                                        skip_group_check=True)
                                    first_mm = False
                        # normalize: y = psy * (1/l), l at col qt*128+64
                        rc = pn.tile([128, 4], F32, tag="rc")
                        for qt in range(4):
                            nc.vector.reciprocal(
                                rc[:, qt:qt + 1],
                                psy[:, qt * 128 + 64:qt * 128 + 65])
                        for qt in range(4):
                            eng = nc.gpsimd if (qt % 2 == 0) else nc.vector
                            eng.tensor_scalar_mul(
                                y_sb[qt][:, h_loc * 64:h_loc * 64 + 64],
                                psy[:, qt * 128:qt * 128 + 64],
                                rc[:, qt:qt + 1])
                    # both heads of this pair done: transpose to yT
                    for qt in range(4):
                        nc.sync.dma_start_transpose(
                            yT[hp][:, qj * 512 + qt * 128:qj * 512 + (qt + 1) * 128],
                            y_sb[qt][:, hp * 128:(hp + 1) * 128])
                # output projection for this tok chunk (PSUM -> DRAM direct)
                for mo in range(8):
                    pps = psC.tile([128, 512], F32, tag="pps")
                    for kt2 in range(2):
                        nc.tensor.matmul(
                            pps[:],
                            wp[:, kt2 * C + mo * 128:kt2 * C + (mo + 1) * 128],
                            yT[kt2][:, qj * 512:(qj + 1) * 512],
                            start=(kt2 == 0), stop=(kt2 == 1))
                    ot = po.tile([128, 512], BF, tag="ot")
                    eng = nc.vector if (mo % 2 == 0) else nc.gpsimd
                    eng.tensor_copy(ot[:], pps[:])
                    nc.sync.dma_start(
                        d_out[:, mo * T + qj * 512:mo * T + (qj + 1) * 512], ot[:])


def _build(reps=1):
    nc = bacc.Bacc("TRN2", target_bir_lowering=False, debug=False)
    d = (
        nc.dram_tensor("xT", [128, CT * T], BF, kind="ExternalInput").ap(),
        nc.dram_tensor("wqk", [128, CT * 512], BF, kind="ExternalInput").ap(),
        nc.dram_tensor("wv", [128, CT * VW], BF, kind="ExternalInput").ap(),
        nc.dram_tensor("wp", [128, 2 * C], BF, kind="ExternalInput").ap(),
        nc.dram_tensor("bqk", [128, 4], F32, kind="ExternalInput").ap(),
        nc.dram_tensor("bv", [128, VW], F32, kind="ExternalInput").ap(),
        nc.dram_tensor("mega", [128, 896], BF, kind="ExternalInput").ap(),
        nc.dram_tensor("outT", [128, 8 * T], BF, kind="ExternalOutput").ap(),
    )
    with tile.TileContext(nc) as tc:
        for rep in range(reps):
            if rep:
                tc.strict_bb_all_engine_barrier()
            _emit(tc, nc, d)
    nc.compile()
    return nc


def _sb(a):
    """[128k, n] -> SBUF layout [128, k*n] (k-tile-major along free dim)."""
    k = a.shape[0] // 128
    return np.ascontiguousarray(
        a.reshape(k, 128, a.shape[1]).transpose(1, 0, 2).reshape(128, -1))


def _bf(a):
    return np.ascontiguousarray(a).astype(ml_dtypes.bfloat16)


def _prep_in_maps(inputs):
    x = np.asarray(inputs["x"], np.float32)
    W_attn = np.asarray(inputs["W_attn"], np.float32)
    b_attn = np.asarray(inputs["b_attn"], np.float32)
    W_proj = np.asarray(inputs["W_proj"], np.float32)

    scale = 1.0 / np.sqrt(HD)
    # mega[p, j] = 1 iff j >= p + 384  (causal mask slices)
    mega = (np.arange(896)[None, :] >= np.arange(128)[:, None] + 384)

    in_maps = []
    for c in range(NCORES):
        b, g = divmod(c, 4)
        heads = [4 * g + i for i in range(HPC)]
        xT = _sb(np.ascontiguousarray(x[b].T))                      # [128, 8*2048]

        wq = [W_attn[:, h * HD:(h + 1) * HD] * scale for h in heads]
        wk = [W_attn[:, C + h * HD:C + (h + 1) * HD] for h in heads]
        wqk = np.concatenate(
            [wq[0], wq[1], wk[0], wk[1], wq[2], wq[3], wk[2], wk[3]], axis=1)
        wqk = _sb(wqk)                                              # [128, 8*512]

        wv = np.zeros((C, VW), np.float32)
        for i, h in enumerate(heads):
            wv[:, i * 65:i * 65 + 64] = W_attn[:, 2 * C + h * HD:2 * C + (h + 1) * HD]
        wv = _sb(wv)                                                # [128, 8*260]

        wp = np.zeros((128, 2 * C), np.float32)
        for kt2 in range(2):
            rows = np.concatenate(
                [W_proj[heads[2 * kt2 + j] * HD:(heads[2 * kt2 + j] + 1) * HD, :]
                 for j in range(2)], axis=0)                        # [128, 1024]
            wp[:, kt2 * C:(kt2 + 1) * C] = rows

        bqk = np.zeros((128, 4), np.float32)
        for i2 in range(2):   # head pair
            for j in range(2):
                h = heads[2 * i2 + j]
                bqk[64 * j:64 * j + 64, 2 * i2] = b_attn[h * HD:(h + 1) * HD] * scale
                bqk[64 * j:64 * j + 64, 2 * i2 + 1] = b_attn[C + h * HD:C + (h + 1) * HD]

        bv = np.zeros(VW, np.float32)
        for i, h in enumerate(heads):
            bv[i * 65:i * 65 + 64] = b_attn[2 * C + h * HD:2 * C + (h + 1) * HD]
            bv[i * 65 + 64] = 1.0
        bv = np.tile(bv[None, :], (128, 1)).astype(np.float32)

        in_maps.append({"xT": _bf(xT), "wqk": _bf(wqk), "wv": _bf(wv),
                        "wp": _bf(wp), "bqk": bqk, "bv": bv,
                        "mega": _bf(mega)})
    return in_maps


def kernel(x, W_attn, b_attn, W_proj, b_proj):
    in_maps = _prep_in_maps(dict(x=x, W_attn=W_attn, b_attn=b_attn,
                                 W_proj=W_proj, b_proj=b_proj))
    if "nc" not in _CACHE:
        _CACHE["nc"] = _build()
    nc = _CACHE["nc"]
    res = run_bass_kernel_spmd(nc, in_maps, core_ids=list(range(NCORES)))

    out = np.zeros((B, T, C), np.float32)
    for c in range(NCORES):
        b = c // 4
        oT = np.asarray(res.results[c]["outT"], np.float32)         # [128, 8*2048]
        oT = oT.reshape(128, 8, T).transpose(1, 0, 2).reshape(C, T)  # [C, T]
        out[b] += oT.T
    out += np.asarray(b_proj, np.float32)[None, None, :]
    return out


# revision 12
# speedup vs baseline: 1.0172x; 1.0172x over previous
"""Causal self-attention (B=2,T=2048,C=1024,H=16) on 8 trn2 NeuronCores.

Sharding: core c handles batch b=c//4 and 4 heads (c%4)*4..+4 (tensor-parallel
over heads x data-parallel over batch).

All matmuls in bf16 (inputs quantized host-side / on-engine; PSUM stays f32):
  stage A: qkT = (Wqk)^T @ x^T  (scale folded into Wq), V = x @ Wv (+ones col)
  stage B: per (head, kt-pair): S^T tile = K' Q -> exp(s-2) -> causal mask
           (tensor_mul with a precomputed triangular mask on diag tiles only)
  AV transposed: y[qt] += es_kt^T-slice @ V_kt  -> psY [128q, 4qt x 65]
           (65th col = ones -> l); y = psY * (1/l) per-partition -> bf16
  yT via DMA xbar transpose; proj: outT = Wp^T @ yT, PSUM -> DRAM direct.
Host sums the 4 per-batch partials, adds b_proj, transposes back.
"""
import sys

sys.path.insert(0, "/opt/trn_rl_repo")

import numpy as np
import ml_dtypes

import concourse.bass as bass
import concourse.mybir as mybir
import concourse.tile as tile
from concourse import bacc
from concourse.bass_utils import run_bass_kernel_spmd

B, T, C, H, HD = 2, 2048, 1024, 16, 64
NCORES = 8
HPC = 4            # heads per core
CT = C // 128      # 8 contraction tiles
TJ = T // 512      # 4 q chunks
TT = T // 128      # 16 tok tiles
VW = HPC * (HD + 1)  # 260: V cols per core incl. ones column per head
F32 = mybir.dt.float32
BF = mybir.dt.bfloat16
EXP = mybir.ActivationFunctionType.Exp

_CACHE = {}


def _emit(tc, nc, d):
    d_xT, d_wqk, d_wv, d_wp, d_bqk, d_bv, d_mega, d_out = d
    with tc.tile_pool(name="const", bufs=1) as pc, \
         tc.tile_pool(name="qk", bufs=1) as pqk, \
         tc.tile_pool(name="vv", bufs=1) as pvv, \
         tc.tile_pool(name="yt", bufs=1) as pyt, \
         tc.tile_pool(name="w_in", bufs=1) as pw, \
         tc.tile_pool(name="x_in", bufs=1) as px, \
         tc.tile_pool(name="fill", bufs=2, space="PSUM") as pfill, \
         tc.tile_pool(name="psS", bufs=2, space="PSUM") as psS, \
         tc.tile_pool(name="psY", bufs=2, space="PSUM") as psY, \
         tc.tile_pool(name="ex", bufs=4) as pex, \
         tc.tile_pool(name="nrm", bufs=4) as pn, \
         tc.tile_pool(name="ysb", bufs=8) as pysb, \
         tc.tile_pool(name="po", bufs=4) as po:
        bqk = pc.tile([128, 4], F32, tag="bqk")
        bv = pc.tile([128, VW], F32, tag="bv")
        mega = pc.tile([128, 896], BF, tag="mega")
        negtwo = pc.tile([128, 1], F32, tag="negtwo")
        warm = pc.tile([128, 512], BF, tag="warm")
        nc.gpsimd.memset(negtwo[:], -2.0)
        nc.gpsimd.memset(warm[:], 0.0)

        qkT = [pqk.tile([128, T], BF, tag=f"qk{i}", name=f"qkT{i}") for i in range(4)]
        V = [pvv.tile([128, VW], BF, tag=f"v{i}", name=f"V{i}") for i in range(TT)]
        yT = [pyt.tile([128, T], BF, tag=f"y{i}", name=f"yT{i}") for i in range(2)]
        wqk = pw.tile([128, CT * 512], BF, tag="wqk")
        wv = pw.tile([128, CT * VW], BF, tag="wv")
        wp = pc.tile([128, 2 * C], BF, tag="wp")
        xT = px.tile([128, CT * T], BF, tag="xT")

        # input DMAs: tj0 slices (interleaved with wqk halves) first so
        # stage A can start ASAP; rest streams behind
        nc.sync.dma_start(wqk[:, :4 * 512], d_wqk[:, :4 * 512])
        for ct in range(CT):
            nc.sync.dma_start(
                xT[:, ct * T:ct * T + 512], d_xT[:, ct * T:ct * T + 512])
        nc.sync.dma_start(bqk[:], d_bqk)
        nc.sync.dma_start(wqk[:, 4 * 512:], d_wqk[:, 4 * 512:])
        nc.sync.dma_start(wv[:], d_wv)
        nc.sync.dma_start(bv[:], d_bv)
        nc.sync.dma_start(mega[:], d_mega)
        for ct in range(CT):
            nc.sync.dma_start(
                xT[:, ct * T + 512:(ct + 1) * T],
                d_xT[:, ct * T + 512:(ct + 1) * T])
        nc.sync.dma_start(wp[:], d_wp)

        # PE p-state warmup during the input-DMA wait (results unused)
        ws = psS.tile([128, 2, 512], F32, tag="s", name="warms")
        for i in range(8):
            nc.tensor.matmul(ws[0:64, 0, :], warm[:, :64], warm[:, :],
                             start=True, stop=True)

        # ---------------- stage A blocks (emitted via filler queue) -------
        def a_qk(tj, mo):
            def emit():
                ps = pfill.tile([128, 512], F32, tag="fill", name=f"psqk{tj}_{mo}")
                for ct in range(CT):
                    nc.tensor.matmul(
                        ps[:],
                        wqk[:, ct * 512 + mo * 128:ct * 512 + (mo + 1) * 128],
                        xT[:, ct * T + tj * 512:ct * T + (tj + 1) * 512],
                        start=(ct == 0), stop=(ct == CT - 1))
                nc.vector.tensor_scalar_add(
                    qkT[mo][:, tj * 512:(tj + 1) * 512], ps[:], bqk[:, mo:mo + 1])
            return emit

        def a_v(tt):
            def emit():
                psv = pfill.tile([128, 512], F32, tag="fill", name=f"psv{tt}")
                for ct in range(CT):
                    nc.tensor.matmul(
                        psv[:, :VW],
                        xT[:, ct * T + tt * 128:ct * T + (tt + 1) * 128],
                        wv[:, ct * VW:(ct + 1) * VW],
                        start=(ct == 0), stop=(ct == CT - 1))
                nc.gpsimd.tensor_add(V[tt][:], psv[:, :VW], bv[:])
            return emit

        def proj(qj, mo):
            def emit():
                pps = pfill.tile([128, 512], F32, tag="fill", name=f"pps{qj}_{mo}")
                for kt2 in range(2):
                    nc.tensor.matmul(
                        pps[:],
                        wp[:, kt2 * C + mo * 128:kt2 * C + (mo + 1) * 128],
                        yT[kt2][:, qj * 512:(qj + 1) * 512],
                        start=(kt2 == 0), stop=(kt2 == 1))
                ot = po.tile([128, 512], BF, tag="ot")
                eng = nc.vector if (mo % 2 == 0) else nc.gpsimd
                eng.tensor_copy(ot[:], pps[:])
                nc.sync.dma_start(
                    d_out[:, mo * T + qj * 512:mo * T + (qj + 1) * 512], ot[:])
            return emit

        # stage A for tj=0 runs up front; the rest interleaves into attention
        for mo in range(4):
            a_qk(0, mo)()
        for tt in range(4):
            a_v(tt)()

        # ---------------- attention + interleaved filler ------------------
        for qj in range(TJ):
            nkt = 4 * qj + 4
            npair = nkt // 2
            filler = []
            if qj + 1 < TJ:
                for mo in range(4):
                    filler.append(a_qk(qj + 1, mo))
                for tt in range(4 * qj + 4, 4 * qj + 8):
                    filler.append(a_v(tt))
            if qj > 0:
                for mo in range(8):
                    filler.append(proj(qj - 1, mo))
            # slots: one after each pair's exp/mask, spread evenly
            nslots = 4 * npair
            spacing = max(1, -(-len(filler) // nslots)) if filler else 0
            slot = 0

            y_sb = [pysb.tile([128, 256], BF, tag="ysb", name=f"ysb{qj}_{q}")
                    for q in range(4)]
            for hp in range(2):
                qt_, kt_ = qkT[2 * hp], qkT[2 * hp + 1]
                for lh in range(2):
                    h_loc = 2 * hp + lh
                    lo, hi = 64 * lh, 64 * lh + 64
                    psy = psY.tile([128, 512], F32, tag="psy")
                    first_mm = True

                    def sc(p):
                        last = (p == npair - 1)
                        off = 256 if last else 0
                        s = psS.tile([128, 2, 512], F32, tag="s")
                        es = pex.tile([128, 2, 512], BF, tag="es")
                        for ki in range(2):
                            kt = 2 * p + ki
                            nc.tensor.matmul(
                                s[:, ki, off:512],
                                kt_[lo:hi, kt * 128:(kt + 1) * 128],
                                qt_[lo:hi, qj * 512 + off:(qj + 1) * 512],
                                start=True, stop=True)
                        return s, es, off

                    tiles = {0: sc(0)}
                    for p in range(npair):
                        if p + 1 < npair:
                            tiles[p + 1] = sc(p + 1)
                        s, es, off = tiles.pop(p)
                        nc.scalar.activation(
                            es[:, :, off:512], s[:, :, off:512], EXP,
                            bias=negtwo[:])
                        for ki in range(2):
                            kt = 2 * p + ki
                            r = kt - 4 * qj
                            if r >= 0:
                                # causal mask: keep k<=q <-> p<=c-128r
                                mo_ = 384 - 128 * r
                                sl = es[:, ki, off:512]
                                eng = nc.vector if (kt % 2 == 0) else nc.gpsimd
                                eng.tensor_mul(
                                    sl, sl, mega[:, mo_ + off:mo_ + 512])
                        # filler block(s) cover the exp latency before AV
                        for _ in range(spacing):
                            if slot < len(filler):
                                filler[slot]()
                                slot += 1
                        # AV (transposed): psy[qt] += es_kt(qt-slice) @ V_kt
                        for qt in range(4):
                            for ki in range(2):
                                kt = 2 * p + ki
                                if kt > 4 * qj + qt:
                                    continue
                                nc.tensor.matmul(
                                    psy[:, qt * 128:qt * 128 + 65],
                                    es[:, ki, qt * 128:(qt + 1) * 128],
                                    V[kt][:, h_loc * 65:h_loc * 65 + 65],
                                    start=first_mm,
                                    stop=(kt == 4 * qj + qt and qt == 3),
                                    skip_group_check=True)
                                first_mm = False
                    # normalize: y = psy * (1/l), l at col qt*128+64
                    rc = pn.tile([128, 4], F32, tag="rc")
                    for qt in range(4):
                        nc.vector.reciprocal(
                            rc[:, qt:qt + 1],
                            psy[:, qt * 128 + 64:qt * 128 + 65])
                    for qt in range(4):
                        eng = nc.gpsimd if (qt % 2 == 0) else nc.vector
                        eng.tensor_scalar_mul(
                            y_sb[qt][:, h_loc * 64:h_loc * 64 + 64],
                            psy[:, qt * 128:qt * 128 + 64],
                            rc[:, qt:qt + 1])
                # both heads of this pair done: transpose to yT
                for qt in range(4):
                    nc.sync.dma_start_transpose(
                        yT[hp][:, qj * 512 + qt * 128:qj * 512 + (qt + 1) * 128],
                        y_sb[qt][:, hp * 128:(hp + 1) * 128])
            # drain leftover filler at end of this qj round
            while slot < len(filler):
                filler[slot]()
                slot += 1
        # final projection
        for mo in range(8):
            proj(3, mo)()


def _build(reps=1):
    nc = bacc.Bacc("TRN2", target_bir_lowering=False, debug=False)
    d = (
        nc.dram_tensor("xT", [128, CT * T], BF, kind="ExternalInput").ap(),
        nc.dram_tensor("wqk", [128, CT * 512], BF, kind="ExternalInput").ap(),
        nc.dram_tensor("wv", [128, CT * VW], BF, kind="ExternalInput").ap(),
        nc.dram_tensor("wp", [128, 2 * C], BF, kind="ExternalInput").ap(),
        nc.dram_tensor("bqk", [128, 4], F32, kind="ExternalInput").ap(),
        nc.dram_tensor("bv", [128, VW], F32, kind="ExternalInput").ap(),
        nc.dram_tensor("mega", [128, 896], BF, kind="ExternalInput").ap(),
        nc.dram_tensor("outT", [128, 8 * T], BF, kind="ExternalOutput").ap(),
    )
    with tile.TileContext(nc) as tc:
        for rep in range(reps):
            if rep:
                tc.strict_bb_all_engine_barrier()
            _emit(tc, nc, d)
    nc.compile()
    return nc


def _sb(a):
    """[128k, n] -> SBUF layout [128, k*n] (k-tile-major along free dim)."""
    k = a.shape[0] // 128
    return np.ascontiguousarray(
        a.reshape(k, 128, a.shape[1]).transpose(1, 0, 2).reshape(128, -1))


def _bf(a):
    return np.ascontiguousarray(a).astype(ml_dtypes.bfloat16)


def _prep_in_maps(inputs):
    x = np.asarray(inputs["x"], np.float32)
    W_attn = np.asarray(inputs["W_attn"], np.float32)
    b_attn = np.asarray(inputs["b_attn"], np.float32)
    W_proj = np.asarray(inputs["W_proj"], np.float32)

    scale = 1.0 / np.sqrt(HD)
    # mega[p, j] = 1 iff j >= p + 384  (causal mask slices)
    mega = (np.arange(896)[None, :] >= np.arange(128)[:, None] + 384)

    in_maps = []
    for c in range(NCORES):
        b, g = divmod(c, 4)
        heads = [4 * g + i for i in range(HPC)]
        xT = _sb(np.ascontiguousarray(x[b].T))                      # [128, 8*2048]

        wq = [W_attn[:, h * HD:(h + 1) * HD] * scale for h in heads]
        wk = [W_attn[:, C + h * HD:C + (h + 1) * HD] for h in heads]
        wqk = np.concatenate(
            [wq[0], wq[1], wk[0], wk[1], wq[2], wq[3], wk[2], wk[3]], axis=1)
        wqk = _sb(wqk)                                              # [128, 8*512]

        wv = np.zeros((C, VW), np.float32)
        for i, h in enumerate(heads):
            wv[:, i * 65:i * 65 + 64] = W_attn[:, 2 * C + h * HD:2 * C + (h + 1) * HD]
        wv = _sb(wv)                                                # [128, 8*260]

        wp = np.zeros((128, 2 * C), np.float32)
        for kt2 in range(2):
            rows = np.concatenate(
                [W_proj[heads[2 * kt2 + j] * HD:(heads[2 * kt2 + j] + 1) * HD, :]
                 for j in range(2)], axis=0)                        # [128, 1024]
            wp[:, kt2 * C:(kt2 + 1) * C] = rows

        bqk = np.zeros((128, 4), np.float32)
        for i2 in range(2):   # head pair
            for j in range(2):
                h = heads[2 * i2 + j]
                bqk[64 * j:64 * j + 64, 2 * i2] = b_attn[h * HD:(h + 1) * HD] * scale
                bqk[64 * j:64 * j + 64, 2 * i2 + 1] = b_attn[C + h * HD:C + (h + 1) * HD]

        bv = np.zeros(VW, np.float32)
        for i, h in enumerate(heads):
            bv[i * 65:i * 65 + 64] = b_attn[2 * C + h * HD:2 * C + (h + 1) * HD]
            bv[i * 65 + 64] = 1.0
        bv = np.tile(bv[None, :], (128, 1)).astype(np.float32)

        in_maps.append({"xT": _bf(xT), "wqk": _bf(wqk), "wv": _bf(wv),
                        "wp": _bf(wp), "bqk": bqk, "bv": bv,
                        "mega": _bf(mega)})
    return in_maps


def kernel(x, W_attn, b_attn, W_proj, b_proj):
    in_maps = _prep_in_maps(dict(x=x, W_attn=W_attn, b_attn=b_attn,
                                 W_proj=W_proj, b_proj=b_proj))
    if "nc" not in _CACHE:
        _CACHE["nc"] = _build()
    nc = _CACHE["nc"]
    res = run_bass_kernel_spmd(nc, in_maps, core_ids=list(range(NCORES)))

    out = np.zeros((B, T, C), np.float32)
    for c in range(NCORES):
        b = c // 4
        oT = np.asarray(res.results[c]["outT"], np.float32)         # [128, 8*2048]
        oT = oT.reshape(128, 8, T).transpose(1, 0, 2).reshape(C, T)  # [C, T]
        out[b] += oT.T
    out += np.asarray(b_proj, np.float32)[None, None, :]
    return out


# revision 17
# speedup vs baseline: 1.1483x; 1.1289x over previous
"""Causal self-attention (B=2,T=2048,C=1024,H=16) on 8 trn2 NeuronCores.

Sharding: core c handles batch b=c//4 and 4 heads (c%4)*4..+4 (tensor-parallel
over heads x data-parallel over batch).

All matmuls in bf16 (inputs quantized host-side / on-engine; PSUM stays f32):
  stage A: qkT = (Wqk)^T @ x^T  (scale folded into Wq), V = x @ Wv (+ones col)
  stage B: per (head, kt-pair): S^T tile = K' Q -> exp(s-2) -> causal mask
           (tensor_mul with a precomputed triangular mask on diag tiles only)
  AV transposed: y[qt] += es_kt^T-slice @ V_kt  -> psY [128q, 4qt x 65]
           (65th col = ones -> l); y = psY * (1/l) per-partition -> bf16
  yT via DMA xbar transpose; proj: outT = Wp^T @ yT, PSUM -> DRAM direct.
Host sums the 4 per-batch partials, adds b_proj, transposes back.
"""
import sys

sys.path.insert(0, "/opt/trn_rl_repo")

import numpy as np
import ml_dtypes

import concourse.bass as bass
import concourse.mybir as mybir
import concourse.tile as tile
from concourse import bacc
from concourse.bass_utils import run_bass_kernel_spmd

B, T, C, H, HD = 2, 2048, 1024, 16, 64
NCORES = 8
HPC = 4            # heads per core
CT = C // 128      # 8 contraction tiles
TJ = T // 512      # 4 q chunks
TT = T // 128      # 16 tok tiles
VW = HPC * (HD + 1)  # 260: V cols per core incl. ones column per head
F32 = mybir.dt.float32
BF = mybir.dt.bfloat16
EXP = mybir.ActivationFunctionType.Exp

_CACHE = {}


def _emit(tc, nc, d):
    d_xT, d_wqk, d_wv, d_wp, d_bqk, d_bv, d_mega, d_out = d
    with tc.tile_pool(name="const", bufs=1) as pc, \
         tc.tile_pool(name="qk", bufs=1) as pqk, \
         tc.tile_pool(name="vv", bufs=1) as pvv, \
         tc.tile_pool(name="yt", bufs=1) as pyt, \
         tc.tile_pool(name="w_in", bufs=1) as pw, \
         tc.tile_pool(name="x_in", bufs=1) as px, \
         tc.tile_pool(name="fill", bufs=2, space="PSUM") as pfill, \
         tc.tile_pool(name="psS", bufs=2, space="PSUM") as psS, \
         tc.tile_pool(name="psY", bufs=2, space="PSUM") as psY, \
         tc.tile_pool(name="ex", bufs=4) as pex, \
         tc.tile_pool(name="nrm", bufs=4) as pn, \
         tc.tile_pool(name="ysb", bufs=8) as pysb, \
         tc.tile_pool(name="po", bufs=4) as po:
        bqk = pc.tile([128, 4], F32, tag="bqk")
        bv = pc.tile([128, VW], F32, tag="bv")
        mega = pc.tile([128, 896], BF, tag="mega")
        negtwo = pc.tile([128, 1], F32, tag="negtwo")
        warm = pc.tile([128, 512], BF, tag="warm")
        nc.gpsimd.memset(negtwo[:], -2.0)
        nc.gpsimd.memset(warm[:], 0.0)

        qkT = [pqk.tile([128, T], BF, tag=f"qk{i}", name=f"qkT{i}") for i in range(4)]
        V = [pvv.tile([128, VW], BF, tag=f"v{i}", name=f"V{i}") for i in range(TT)]
        yT = [pyt.tile([128, T], BF, tag=f"y{i}", name=f"yT{i}") for i in range(2)]
        wqk = pw.tile([128, CT * 512], BF, tag="wqk")
        wv = pw.tile([128, CT * VW], BF, tag="wv")
        wp = pc.tile([128, 2 * C], BF, tag="wp")
        xT = px.tile([128, CT * T], BF, tag="xT")

        # input DMAs: xT is tj-major [tj, ct, 512], wqk is mo-major
        # [mo, ct, 128] so the first qk group only needs 2 DMAs
        nc.sync.dma_start(wqk[:, :1024], d_wqk[:, :1024])
        nc.sync.dma_start(xT[:, :4096], d_xT[:, :4096])
        for mo in range(1, 4):
            nc.sync.dma_start(
                wqk[:, mo * 1024:(mo + 1) * 1024],
                d_wqk[:, mo * 1024:(mo + 1) * 1024])
        nc.sync.dma_start(bqk[:], d_bqk)
        nc.sync.dma_start(wv[:], d_wv)
        nc.sync.dma_start(bv[:], d_bv)
        nc.sync.dma_start(mega[:], d_mega)
        for tj in range(1, TJ):
            nc.sync.dma_start(
                xT[:, tj * 4096:(tj + 1) * 4096],
                d_xT[:, tj * 4096:(tj + 1) * 4096])
        nc.sync.dma_start(wp[:], d_wp)

        # PE p-state warmup during the input-DMA wait (results unused)
        ws = psS.tile([128, 2, 512], F32, tag="s", name="warms")
        for i in range(8):
            nc.tensor.matmul(ws[0:64, 0, :], warm[:, :64], warm[:, :],
                             start=True, stop=True)

        # ---------------- stage A blocks (emitted via filler queue) -------
        def a_qk(tj, mo):
            def emit():
                ps = pfill.tile([128, 512], F32, tag="fill", name=f"psqk{tj}_{mo}")
                for ct in range(CT):
                    nc.tensor.matmul(
                        ps[:],
                        wqk[:, mo * 1024 + ct * 128:mo * 1024 + (ct + 1) * 128],
                        xT[:, tj * 4096 + ct * 512:tj * 4096 + (ct + 1) * 512],
                        start=(ct == 0), stop=(ct == CT - 1))
                nc.vector.tensor_scalar_add(
                    qkT[mo][:, tj * 512:(tj + 1) * 512], ps[:], bqk[:, mo:mo + 1])
            return emit

        def a_v(tt):
            tj, ti = divmod(tt, 4)

            def emit():
                psv = pfill.tile([128, 512], F32, tag="fill", name=f"psv{tt}")
                for ct in range(CT):
                    nc.tensor.matmul(
                        psv[:, :VW],
                        xT[:, tj * 4096 + ct * 512 + ti * 128:
                           tj * 4096 + ct * 512 + (ti + 1) * 128],
                        wv[:, ct * VW:(ct + 1) * VW],
                        start=(ct == 0), stop=(ct == CT - 1))
                nc.gpsimd.tensor_add(V[tt][:], psv[:, :VW], bv[:])
            return emit

        def proj(qj, mo):
            def emit():
                pps = pfill.tile([128, 512], F32, tag="fill", name=f"pps{qj}_{mo}")
                for kt2 in range(2):
                    nc.tensor.matmul(
                        pps[:],
                        wp[:, kt2 * C + mo * 128:kt2 * C + (mo + 1) * 128],
                        yT[kt2][:, qj * 512:(qj + 1) * 512],
                        start=(kt2 == 0), stop=(kt2 == 1))
                ot = po.tile([128, 512], BF, tag="ot")
                eng = nc.vector if (mo % 2 == 0) else nc.gpsimd
                eng.tensor_copy(ot[:], pps[:])
                nc.sync.dma_start(
                    d_out[:, mo * T + qj * 512:mo * T + (qj + 1) * 512], ot[:])
            return emit

        # stage A for tj=0 runs up front; the rest interleaves into attention
        for mo in range(4):
            a_qk(0, mo)()
        for tt in range(4):
            a_v(tt)()

        # ---------------- attention sections + interleaved filler ---------
        ysb_tiles = {}

        def section(qj, hp, filler):
            nkt = 4 * qj + 4
            npair = nkt // 2
            nslots = 2 * npair
            spacing = max(1, -(-len(filler) // nslots)) if filler else 0
            slot = 0
            if qj not in ysb_tiles:
                ysb_tiles[qj] = [
                    pysb.tile([128, 256], BF, tag="ysb", name=f"ysb{qj}_{q}")
                    for q in range(4)]
            y_sb = ysb_tiles[qj]
            qt_, kt_ = qkT[2 * hp], qkT[2 * hp + 1]
            for lh in range(2):
                h_loc = 2 * hp + lh
                lo, hi = 64 * lh, 64 * lh + 64
                psy = psY.tile([128, 512], F32, tag="psy")
                first_mm = True

                def sc(p):
                    last = (p == npair - 1)
                    off = 256 if last else 0
                    s = psS.tile([128, 2, 512], F32, tag="s")
                    es = pex.tile([128, 2, 512], BF, tag="es")
                    for ki in range(2):
                        kt = 2 * p + ki
                        nc.tensor.matmul(
                            s[:, ki, off:512],
                            kt_[lo:hi, kt * 128:(kt + 1) * 128],
                            qt_[lo:hi, qj * 512 + off:(qj + 1) * 512],
                            start=True, stop=True)
                    return s, es, off

                tiles = {0: sc(0)}
                for p in range(npair):
                    if p + 1 < npair:
                        tiles[p + 1] = sc(p + 1)
                    s, es, off = tiles.pop(p)
                    nc.scalar.activation(
                        es[:, :, off:512], s[:, :, off:512], EXP,
                        bias=negtwo[:])
                    for ki in range(2):
                        kt = 2 * p + ki
                        r = kt - 4 * qj
                        if r >= 0:
                            # causal mask: keep k<=q <-> p<=c-128r
                            mo_ = 384 - 128 * r
                            sl = es[:, ki, off:512]
                            eng = nc.vector if (kt % 2 == 0) else nc.gpsimd
                            eng.tensor_mul(
                                sl, sl, mega[:, mo_ + off:mo_ + 512])
                    # filler block(s) cover the exp latency before AV
                    for _ in range(spacing):
                        if slot < len(filler):
                            filler[slot]()
                            slot += 1
                    # AV (transposed): psy[qt] += es_kt(qt-slice) @ V_kt
                    for qt in range(4):
                        for ki in range(2):
                            kt = 2 * p + ki
                            if kt > 4 * qj + qt:
                                continue
                            nc.tensor.matmul(
                                psy[:, qt * 128:qt * 128 + 65],
                                es[:, ki, qt * 128:(qt + 1) * 128],
                                V[kt][:, h_loc * 65:h_loc * 65 + 65],
                                start=first_mm,
                                stop=(kt == 4 * qj + qt),
                                skip_group_check=True)
                            first_mm = False
                # normalize: y = psy * (1/l), l at col qt*128+64
                rc = pn.tile([128, 4], F32, tag="rc")
                for qt in range(4):
                    nc.vector.reciprocal(
                        rc[:, qt:qt + 1],
                        psy[:, qt * 128 + 64:qt * 128 + 65])
                for qt in range(4):
                    eng = nc.gpsimd if (qt % 2 == 0) else nc.vector
                    eng.tensor_scalar_mul(
                        y_sb[qt][:, h_loc * 64:h_loc * 64 + 64],
                        psy[:, qt * 128:qt * 128 + 64],
                        rc[:, qt:qt + 1])
            # both heads of this pair done: transpose to yT
            for qt in range(4):
                nc.sync.dma_start_transpose(
                    yT[hp][:, qj * 512 + qt * 128:qj * 512 + (qt + 1) * 128],
                    y_sb[qt][:, hp * 128:(hp + 1) * 128])
            # drain leftover filler
            while slot < len(filler):
                filler[slot]()
                slot += 1

        A = {tj: [a_qk(tj, mo) for mo in range(4)] +
                 [a_v(tt) for tt in range(4 * tj, 4 * tj + 4)]
             for tj in range(1, TJ)}
        P = {qj: [proj(qj, mo) for mo in range(8)] for qj in range(TJ)}
        # diagonal tail: (2,h0)[A3] (2,h1)[proj1] (3,h0)[proj2... proj2 needs
        # (2,h1); order: (2,h0) (2,h1) (3,h0) (3,h1)
        section(0, 0, A[1][:4])
        section(0, 1, A[1][4:])
        section(1, 0, A[2][:4])
        section(1, 1, A[2][4:] + P[0][:4])
        section(2, 0, P[0][4:] + A[3])
        section(2, 1, P[1])
        section(3, 0, P[2][:5])
        section(3, 1, P[2][5:])
        for mo in range(8):
            P[3][mo]()


def _build(reps=1):
    nc = bacc.Bacc("TRN2", target_bir_lowering=False, debug=False)
    d = (
        nc.dram_tensor("xT", [128, CT * T], BF, kind="ExternalInput").ap(),
        nc.dram_tensor("wqk", [128, CT * 512], BF, kind="ExternalInput").ap(),
        nc.dram_tensor("wv", [128, CT * VW], BF, kind="ExternalInput").ap(),
        nc.dram_tensor("wp", [128, 2 * C], BF, kind="ExternalInput").ap(),
        nc.dram_tensor("bqk", [128, 4], F32, kind="ExternalInput").ap(),
        nc.dram_tensor("bv", [128, VW], F32, kind="ExternalInput").ap(),
        nc.dram_tensor("mega", [128, 896], BF, kind="ExternalInput").ap(),
        nc.dram_tensor("outT", [128, 8 * T], BF, kind="ExternalOutput").ap(),
    )
    with tile.TileContext(nc) as tc:
        for rep in range(reps):
            if rep:
                tc.strict_bb_all_engine_barrier()
            _emit(tc, nc, d)
    nc.compile()
    return nc


def _sb(a):
    """[128k, n] -> SBUF layout [128, k*n] (k-tile-major along free dim)."""
    k = a.shape[0] // 128
    return np.ascontiguousarray(
        a.reshape(k, 128, a.shape[1]).transpose(1, 0, 2).reshape(128, -1))


def _bf(a):
    return np.ascontiguousarray(a).astype(ml_dtypes.bfloat16)


def _prep_in_maps(inputs):
    x = np.asarray(inputs["x"], np.float32)
    W_attn = np.asarray(inputs["W_attn"], np.float32)
    b_attn = np.asarray(inputs["b_attn"], np.float32)
    W_proj = np.asarray(inputs["W_proj"], np.float32)

    scale = 1.0 / np.sqrt(HD)
    # mega[p, j] = 1 iff j >= p + 384  (causal mask slices)
    mega = (np.arange(896)[None, :] >= np.arange(128)[:, None] + 384)

    in_maps = []
    for c in range(NCORES):
        b, g = divmod(c, 4)
        heads = [4 * g + i for i in range(HPC)]
        xT = _sb(np.ascontiguousarray(x[b].T))                      # [128, 8*2048]
        # tj-major: [128, tj, ct, 512] so one DMA covers a whole tj chunk
        xT = np.ascontiguousarray(
            xT.reshape(128, CT, TJ, 512).transpose(0, 2, 1, 3).reshape(128, -1))

        wq = [W_attn[:, h * HD:(h + 1) * HD] * scale for h in heads]
        wk = [W_attn[:, C + h * HD:C + (h + 1) * HD] for h in heads]
        wqk = np.concatenate(
            [wq[0], wq[1], wk[0], wk[1], wq[2], wq[3], wk[2], wk[3]], axis=1)
        wqk = _sb(wqk)                                              # [128, 8*512]
        # mo-major: [128, mo, ct, 128]
        wqk = np.ascontiguousarray(
            wqk.reshape(128, CT, 4, 128).transpose(0, 2, 1, 3).reshape(128, -1))

        wv = np.zeros((C, VW), np.float32)
        for i, h in enumerate(heads):
            wv[:, i * 65:i * 65 + 64] = W_attn[:, 2 * C + h * HD:2 * C + (h + 1) * HD]
        wv = _sb(wv)                                                # [128, 8*260]

        wp = np.zeros((128, 2 * C), np.float32)
        for kt2 in range(2):
            rows = np.concatenate(
                [W_proj[heads[2 * kt2 + j] * HD:(heads[2 * kt2 + j] + 1) * HD, :]
                 for j in range(2)], axis=0)                        # [128, 1024]
            wp[:, kt2 * C:(kt2 + 1) * C] = rows

        bqk = np.zeros((128, 4), np.float32)
        for i2 in range(2):   # head pair
            for j in range(2):
                h = heads[2 * i2 + j]
                bqk[64 * j:64 * j + 64, 2 * i2] = b_attn[h * HD:(h + 1) * HD] * scale
                bqk[64 * j:64 * j + 64, 2 * i2 + 1] = b_attn[C + h * HD:C + (h + 1) * HD]

        bv = np.zeros(VW, np.float32)
        for i, h in enumerate(heads):
            bv[i * 65:i * 65 + 64] = b_attn[2 * C + h * HD:2 * C + (h + 1) * HD]
            bv[i * 65 + 64] = 1.0
        bv = np.tile(bv[None, :], (128, 1)).astype(np.float32)

        in_maps.append({"xT": _bf(xT), "wqk": _bf(wqk), "wv": _bf(wv),
                        "wp": _bf(wp), "bqk": bqk, "bv": bv,
                        "mega": _bf(mega)})
    return in_maps


def kernel(x, W_attn, b_attn, W_proj, b_proj):
    in_maps = _prep_in_maps(dict(x=x, W_attn=W_attn, b_attn=b_attn,
                                 W_proj=W_proj, b_proj=b_proj))
    if "nc" not in _CACHE:
        _CACHE["nc"] = _build()
    nc = _CACHE["nc"]
    res = run_bass_kernel_spmd(nc, in_maps, core_ids=list(range(NCORES)))

    out = np.zeros((B, T, C), np.float32)
    for c in range(NCORES):
        b = c // 4
        oT = np.asarray(res.results[c]["outT"], np.float32)         # [128, 8*2048]
        oT = oT.reshape(128, 8, T).transpose(1, 0, 2).reshape(C, T)  # [C, T]
        out[b] += oT.T
    out += np.asarray(b_proj, np.float32)[None, None, :]
    return out
